# revision 23
# baseline (speedup 1.0000x reference)
"""MultiHeadAttention Trainium2 kernel (8 NeuronCores, data-parallel over batch).

Contract: kernel(**inputs) takes the FULL inputs from setup_inputs() and
returns the FULL [8, 512, 1024] output. Batch element c runs on NeuronCore c
(B == n_cores == 8); no collectives.

Per-core computation (batch b, S=512, D=1024, H=16, Dk=64), all matmul
operands fp16 (same 10-bit mantissa as fp32r, half the HBM traffic):
  QT = (w_q/8)^T-proj of query^T  -> [D, S]  (head h rows h*64..h*64+63)
  KT likewise (unscaled)          -> [D, S]
  V  = natural value proj         -> [S, D]  (stored with a ones column per head)
  per head: scoresT[k,q'] = KT_h-block^T @ QT_h   (q' = reversed query index)
            attnT = exp(scoresT) * emt[h]         (emt = host exp(rel-bias) *
                    {0,1} mask -- fp16*fp16 runs DVE at 2x, masking is exact)
            ctxT[65,S] = [V_h | 1]^T @ attnT      (row 64 = softmax denominators)
            ctxT_norm = ctxT[0:64] * bcast(1/denom)   (GpSimd partition bcast)
  out_rev[q', e] = ctxT_norm^T-chunks @ w_o^T + b_o ; host un-reverses rows.

Performance structure:
  - fp16 operands halve HBM reads; exp(amt) multiply instead of amt add keeps
    DVE in 2x fast mode and off the PSUM port
  - few, large DMAs (each DMA costs ~2us of ring latency); early-critical
    tensors (qT/kT/wqk0/emt pair0) ride the scalar HWDGE ring in parallel
    with vT/wv on the sync ring
  - per-chunk order [Q-proj, K-proj, ctx(h-2), ctx(h-1), scores(h), scores(h+1)]
    covers the K bias-add and exp latencies with PE work
  - warm-up + mid-stream dummy matmuls keep the HAM clock-gate at 8/8 (idle
    gaps re-throttle the PE to half clock for >13us)
"""
import numpy as np

import concourse.bass as bass
import concourse.tile as tile
from concourse import bacc, mybir
from concourse.bass_utils import run_bass_kernel_spmd

S = 512
D = 1024
H = 16
DK = 64
N_CORES = 8
NCH = D // 128  # 8 d-model chunks of 128
SB = S // 128   # 4 seq blocks of 128
F32 = mybir.dt.float32
F16 = mybir.dt.float16

_CACHE = {}


def _build_program():
    nc = bacc.Bacc("TRN2", target_bir_lowering=False, debug=False,
                   num_devices=N_CORES)

    # Per-core DRAM inputs, host-laid-out so every DMA is contiguous per
    # partition and as large as its consumption grain allows.
    qT = nc.dram_tensor("qT", [128, NCH, S], F16, kind="ExternalInput").ap()
    kT = nc.dram_tensor("kT", [128, NCH, S], F16, kind="ExternalInput").ap()
    vT = nc.dram_tensor("vT", [128, NCH, S], F16, kind="ExternalInput").ap()
    # emt[pair_of_heads][p][sub][kb-pair][2*S]
    emt = nc.dram_tensor("emt", [NCH, 128, 2, 2, 2 * S], F16,
                         kind="ExternalInput").ap()
    wqk = nc.dram_tensor("wqk", [NCH, 128, 2, D], F16, kind="ExternalInput").ap()
    wv = nc.dram_tensor("wv", [128, 2, NCH, S], F16, kind="ExternalInput").ap()
    wo = nc.dram_tensor("wo", [128, NCH, D], F16, kind="ExternalInput").ap()
    bqk = nc.dram_tensor("bqk", [128, 2, NCH], F32, kind="ExternalInput").ap()
    bvo = nc.dram_tensor("bvo", [1, 2, D], F16, kind="ExternalInput").ap()
    out = nc.dram_tensor("out", [S, D], F16, kind="ExternalOutput").ap()
    out3 = out.rearrange("(sb p) e -> sb p e", p=128)  # [4, 128, 1024]

    from contextlib import ExitStack

    with tile.TileContext(nc) as tc, ExitStack() as ctx:
        singles = ctx.enter_context(tc.tile_pool(name="singles", bufs=1))
        wpool = ctx.enter_context(tc.tile_pool(name="wpool", bufs=5))
        amtpool = ctx.enter_context(tc.tile_pool(name="amtpool", bufs=5))
        attnpool = ctx.enter_context(tc.tile_pool(name="attnpool", bufs=12))
        rcpool = ctx.enter_context(tc.tile_pool(name="rcpool", bufs=6))
        rbpool = ctx.enter_context(tc.tile_pool(name="rbpool", bufs=2))
        outpool = ctx.enter_context(tc.tile_pool(name="outpool", bufs=2))
        ps_proj = ctx.enter_context(tc.tile_pool(name="ps_proj", bufs=2, space="PSUM"))
        ps_sc = ctx.enter_context(tc.tile_pool(name="ps_sc", bufs=2, space="PSUM"))
        ps_ctx = ctx.enter_context(tc.tile_pool(name="ps_ctx", bufs=2, space="PSUM"))

        # early-critical small/medium loads on the scalar HWDGE ring, in
        # parallel with the sync ring's vT/wv stream
        bqk_sb = singles.tile([128, 2, NCH], F32, tag="bqk")
        bvo_sb = singles.tile([1, 2, D], F16, tag="bvo")
        nc.scalar.dma_start(out=bqk_sb, in_=bqk)
        nc.scalar.dma_start(out=bvo_sb, in_=bvo)
        bq_sb, bk_sb = bqk_sb[:, 0, :], bqk_sb[:, 1, :]
        bvr_sb, bor_sb = bvo_sb[:, 0, :], bvo_sb[:, 1, :]
        qT_sb = singles.tile([128, NCH, S], F16, tag="qT")
        kT_sb = singles.tile([128, NCH, S], F16, tag="kT")
        nc.scalar.dma_start(out=qT_sb, in_=qT)
        nc.scalar.dma_start(out=kT_sb, in_=kT)
        wqk_sb = [None] * NCH
        emt_tiles = [None] * NCH

        def queue_wqk(i, eng):
            t = wpool.tile([128, 2, D], F16, tag="w")
            eng.dma_start(out=t, in_=wqk[i])
            wqk_sb[i] = t

        def queue_emt(pr, eng):
            t = amtpool.tile([128, 2, 2, 2 * S], F16, tag="amt")
            eng.dma_start(out=t, in_=emt[pr])
            emt_tiles[pr] = t

        queue_wqk(0, nc.scalar)
        queue_emt(0, nc.scalar)

        ones_sb = singles.tile([1, 128], F16, tag="ones")
        nc.vector.memset(ones_sb, 1.0)

        # HAM warm-up: throwaway matmuls while the input DMAs stream, so the
        # PE clock-gate ramp starts counting immediately.
        for _ in range(56):
            pd = ps_proj.tile([128, 512], F32, tag="proj")
            nc.tensor.matmul(pd[:, :128], lhsT=ones_sb, rhs=ones_sb,
                             start=True, stop=True)

        # ---- bulk loads on the sync ring, in exact consumption order ----
        vT_sb = singles.tile([128, NCH, S], F16, tag="vT")
        nc.sync.dma_start(out=vT_sb, in_=vT)
        wv_sb = singles.tile([128, 2, NCH, S], F16, tag="wv")
        for eh in range(2):
            nc.sync.dma_start(out=wv_sb[:, eh, :, :], in_=wv[:, eh, :, :])
        for i in range(1, NCH):
            queue_wqk(i, nc.sync)
            queue_emt(i, nc.sync)
        wo_sb = singles.tile([128, NCH, D], F16, tag="wo")
        nc.sync.dma_start(out=wo_sb, in_=wo)

        # big persistent activations
        QT_sb = singles.tile([128, NCH, S], F16, tag="QT")
        KT_sb = singles.tile([128, NCH, S], F16, tag="KT")
        # V with a ones column appended per head: [128, sb, 16*65]
        V_sb = singles.tile([128, SB, H * (DK + 1)], F16, tag="V")
        ctxT_sb = singles.tile([128, NCH, S], F16, tag="ctxT")
        ones_col = singles.tile([128, H], F16, tag="ones_col")
        nc.vector.memset(ones_col, 1.0)

        # ---- V projection: V[s, e] = vT^T @ wv + b_v ----
        # eh-outer so the first half only needs wv's first 1MB to have landed
        for sb in range(SB):
            v_heads = V_sb[:, sb, :].rearrange("p (h c) -> p h c", c=DK + 1)
            nc.scalar.copy(v_heads[:, :, DK], ones_col)
        for eh in range(2):
            for sb in range(SB):
                pv = ps_proj.tile([128, 512], F32, tag="proj")
                for dc in range(NCH):
                    nc.tensor.matmul(
                        pv,
                        lhsT=vT_sb[:, dc, sb * 128:(sb + 1) * 128],
                        rhs=wv_sb[:, eh, dc, :],
                        start=(dc == 0), stop=False,
                    )
                nc.tensor.matmul(
                    pv, lhsT=ones_sb,
                    rhs=bvr_sb[:, eh * 512:(eh + 1) * 512],
                    start=False, stop=True,
                )
                v_heads = V_sb[:, sb, :].rearrange("p (h c) -> p h c", c=DK + 1)
                nc.scalar.copy(
                    v_heads[:, 8 * eh:8 * eh + 8, 0:DK],
                    pv.rearrange("p (h d) -> p h d", d=DK),
                )

        # mid-stream dummies: keep the PE (and the HAM activity window) busy
        # while qT/kT/wqk0/emt0 finish streaming in
        for _ in range(12):
            pd = ps_proj.tile([128, 512], F32, tag="proj")
            nc.tensor.matmul(pd[:, :128], lhsT=ones_sb, rhs=ones_sb,
                             start=True, stop=True)

        # ---- interleaved Q/K projection chunks + attention heads ----
        def emit_scores(h):
            i, p0 = h // 2, (h % 2) * 64
            emt_h = emt_tiles[i][:, h % 2, :, :]
            QT_h = QT_sb[p0:p0 + 64, i, :]
            attn_tiles = []
            for pair in range(2):
                ps = ps_sc.tile([128, 2 * S], F32, tag="pair")
                for j in range(2):
                    kb = 2 * pair + j
                    nc.tensor.matmul(
                        ps[:, j * 512:(j + 1) * 512],
                        lhsT=KT_sb[p0:p0 + 64, i, kb * 128:(kb + 1) * 128],
                        rhs=QT_h, start=True, stop=True,
                    )
                ate = attnpool.tile([128, 2 * S], F16, tag="attn")
                nc.scalar.activation(ate, ps, mybir.ActivationFunctionType.Exp)
                at = attnpool.tile([128, 2 * S], F16, tag="attn")
                nc.vector.tensor_mul(at, ate, emt_h[:, pair, :])
                attn_tiles.append(at)
            return attn_tiles

        def emit_ctx(h, attn_tiles):
            i, p0 = h // 2, (h % 2) * 64
            pc = ps_ctx.tile([DK + 1, 512], F32, tag="ctx")
            for kb in range(SB):
                nc.tensor.matmul(
                    pc, lhsT=V_sb[:, kb, h * 65:(h + 1) * 65],
                    rhs=attn_tiles[kb // 2][:, (kb % 2) * 512:(kb % 2 + 1) * 512],
                    start=(kb == 0), stop=(kb == SB - 1),
                )
            # custom-DVE reciprocal can't read PSUM on HW; stage sums in SBUF
            sums_sb = rcpool.tile([1, 512], F32, tag="recip")
            nc.vector.tensor_scalar_add(sums_sb, pc[DK:DK + 1, :], 0.0)
            recip_f32 = rcpool.tile([1, 512], F32, tag="recip")
            nc.vector.reciprocal_approx_fast(out=recip_f32, in_=sums_sb)
            # broadcast 1/denom across 64 partitions on GpSimd (whose queue
            # does nothing else, so broadcasts never queue behind DMAs)
            r_sb = rbpool.tile([64, 512], F32, tag="rbc")
            nc.gpsimd.partition_broadcast(r_sb, recip_f32, channels=64)
            nc.vector.tensor_mul(ctxT_sb[p0:p0 + 64, i, :], pc[0:DK, :], r_sb)

        # Per-chunk emission order: the two context groups run while K's
        # bias-add drains, and each head's exp+mask gets a full chunk of PE
        # work as cover before its context matmuls need it.
        pending = []  # [(head, attn_tiles)] awaiting context matmuls
        for i in range(NCH):  # e-chunk i covers heads 2i, 2i+1
            pq = ps_proj.tile([128, 512], F32, tag="proj")
            for dc in range(NCH):
                nc.tensor.matmul(
                    pq, lhsT=wqk_sb[i][:, 0, dc * 128:(dc + 1) * 128],
                    rhs=qT_sb[:, dc, :],
                    start=(dc == 0), stop=(dc == NCH - 1),
                )
            nc.scalar.add(QT_sb[:, i, :], pq, bq_sb[:, i:i + 1])
            pk = ps_proj.tile([128, 512], F32, tag="proj")
            for dc in range(NCH):
                nc.tensor.matmul(
                    pk, lhsT=wqk_sb[i][:, 1, dc * 128:(dc + 1) * 128],
                    rhs=kT_sb[:, dc, :],
                    start=(dc == 0), stop=(dc == NCH - 1),
                )
            nc.scalar.add(KT_sb[:, i, :], pk, bk_sb[:, i:i + 1])

            for p in pending:
                emit_ctx(*p)
            pending = []
            for sub in range(2):
                h = 2 * i + sub
                pending.append((h, emit_scores(h)))
        for p in pending:
            emit_ctx(*p)

        # ---- output projection: out_rev[q', e] = ctxT^T @ wo + b_o ----
        for sb in range(SB):
            po = ps_sc.tile([128, 2 * S], F32, tag="pair")
            for eh in range(2):
                half = po[:, eh * 512:(eh + 1) * 512]
                for ch in range(NCH):
                    nc.tensor.matmul(
                        half, lhsT=ctxT_sb[:, ch, sb * 128:(sb + 1) * 128],
                        rhs=wo_sb[:, ch, eh * 512:(eh + 1) * 512],
                        start=(ch == 0), stop=False,
                    )
                nc.tensor.matmul(
                    half, lhsT=ones_sb,
                    rhs=bor_sb[:, eh * 512:(eh + 1) * 512],
                    start=False, stop=True,
                )
            osb = outpool.tile([128, D], F16, tag="out")
            nc.scalar.copy(osb, po)
            nc.sync.dma_start(out=out3[sb], in_=osb)

    nc.compile()
    return nc


def _prep_inputs(query, key, value, mask, w_q, b_q, w_k, b_k, w_v, b_v,
                 w_o, b_o, rel_bias):
    query = np.asarray(query, np.float32)
    key = np.asarray(key, np.float32)
    value = np.asarray(value, np.float32)
    mask = np.asarray(mask)
    w_q = np.asarray(w_q, np.float32)
    w_k = np.asarray(w_k, np.float32)
    w_v = np.asarray(w_v, np.float32)
    w_o = np.asarray(w_o, np.float32)
    b_q = np.asarray(b_q, np.float32)
    b_k = np.asarray(b_k, np.float32)
    b_v = np.asarray(b_v, np.float32)
    b_o = np.asarray(b_o, np.float32)
    rel_bias = np.asarray(rel_bias, np.float32)

    def chunked_T(w):
        # wc[i, p, dc*128+e] = w.T[dc*128+p, i*128+e]: each proj chunk i is
        # one contiguous [128, 1024] block
        wt = np.ascontiguousarray(w.T).reshape(NCH, 128, NCH, 128)
        return wt.transpose(2, 1, 0, 3).reshape(NCH, 128, D)

    def part_major(xT_cols):
        # [1024, 512] -> [128, 8, 512] with row dc*128+p at [p, dc]
        return np.ascontiguousarray(
            xT_cols.reshape(NCH, 128, S).transpose(1, 0, 2).astype(np.float16))

    wqk = np.stack([chunked_T(w_q / 8.0), chunked_T(w_k)], axis=2)
    shared = {
        "wqk": np.ascontiguousarray(wqk.astype(np.float16)),
        "wv": np.ascontiguousarray(
            w_v.T.reshape(NCH, 128, 2, S).transpose(1, 2, 0, 3).astype(np.float16)),
        "wo": np.ascontiguousarray(
            w_o.T.reshape(NCH, 128, D).transpose(1, 0, 2).astype(np.float16)),
        "bqk": np.ascontiguousarray(np.stack(
            [(b_q / 8.0).reshape(NCH, 128).T, b_k.reshape(NCH, 128).T], axis=1)),
        "bvo": np.stack([b_v, b_o]).reshape(1, 2, D).astype(np.float16),
    }

    # biasT_rev[h, k, q'] = rel_bias[k + q', h]; emt = exp(bias) * mask01
    idx = np.arange(S)[:, None] + np.arange(S)[None, :]  # [k, q'] in [0, 1022]
    ebias_t = np.exp(rel_bias)[idx]        # [S, S, H]
    ebias_t = np.ascontiguousarray(ebias_t.transpose(2, 0, 1))  # [H, k, q']

    in_maps = []
    for c in range(N_CORES):
        # maskT_rev[k, q'] multiplicative: mask[c, 0, 511-q', k] in {0, 1}
        m01 = mask[c, 0][::-1, :].T.astype(np.float32)   # [k, q']
        a = (ebias_t * m01[None]).astype(np.float16)     # [H, k, q']
        # [h=2pr+sub, k=(2pair+j)*128+p, q] -> [pr, p, sub, pair, j*512+q]
        a = a.reshape(NCH, 2, 2, 2, 128, S).transpose(0, 4, 1, 2, 3, 5)
        im = dict(shared)
        im["qT"] = part_major(query[c].T[:, ::-1])
        im["kT"] = part_major(key[c].T)
        im["vT"] = part_major(value[c].T)
        im["emt"] = np.ascontiguousarray(a).reshape(NCH, 128, 2, 2, 2 * S)
        in_maps.append(im)
    return in_maps


def kernel(query, key, value, mask, w_q, b_q, w_k, b_k, w_v, b_v, w_o, b_o,
           rel_bias, _run_opts=None):
    if "nc" not in _CACHE:
        _CACHE["nc"] = _build_program()
    nc = _CACHE["nc"]
    in_maps = _prep_inputs(query, key, value, mask, w_q, b_q, w_k, b_k,
                           w_v, b_v, w_o, b_o, rel_bias)
    opts = _run_opts or {}
    res = run_bass_kernel_spmd(nc, in_maps, list(range(N_CORES)), **opts)
    out = np.stack([res.results[c]["out"][::-1, :] for c in range(N_CORES)])
    if _run_opts is not None:
        _CACHE["last_result"] = res
    return out.astype(np.float32)
